# revision 18
# baseline (speedup 1.0000x reference)
"""LocalAttention3D Trainium2 kernel — v3: bias-free exp + f-folded stationaries.

Factorization (per core: batch b, jk-slice of 1024):
  e_h[lm,jk] = exp(a_h*G[lm,jk] + c_h[lm] - 33) = f_h[lm] * E_h[lm,jk]
    with E_h = exp(a_h*G - 33)   (uniform scale/bias -> ACT pair-batches banks)
         f_h = exp(c_h)          (folded into BOf / XTf host-side constants)
  Z_h[l,jk]  = sum_m f_h*E_h     (MMZ: per-head M=16 col-strip, 4 heads conc.)
  zib_h      = |wv_h| / Z_h      (ACT: Ln then Exp(-x + ln|wv_h|))
  zb_h[lm,jk]= broadcast_m zib   (SEL: K=16 row-strip matmul, head-pairs conc.)
  p_h        = E_h * zb_h        (DVE tensor_mul, pair-batched from PSUM)
  out[d,jk]  = sum_{h,t} (sign(wv_h)*f_h*XT)^T p_h  + 64*sum_h bv_h
"""

import math
import sys

sys.path.insert(0, "/opt/trn_rl_repo")

import numpy as np
import ml_dtypes

import bass_rust
import concourse.bass as bass
import concourse.tile as tile
from concourse import mybir
from concourse.bass_utils import run_bass_kernel_spmd

BF16 = ml_dtypes.bfloat16

B, D, HW = 2, 96, 64 * 64
NH = 4
NCORES = 8
SLC = 1024            # jk columns per core
JKC = 512             # jk columns per chunk
NJC = SLC // JKC      # 2 chunks
NT = HW // 128        # 32 lm-tiles
NOG = NT // 8         # 4 octet groups of 8 lm-tiles
SCALE = 1.0 / math.sqrt(32.0)
EBIAS = -33.0


def _split_excess_waits(nc, max_waits=1):
    ctr = 0
    for f in nc.m.functions:
        for blk in f.blocks:
            insts = blk.instructions
            out = []
            changed = False
            for ins in insts:
                try:
                    si = ins.sync_info
                except Exception:
                    si = None
                if si is not None and len(si.on_wait) > max_waits:
                    waits = list(si.on_wait)
                    for w in waits[:-max_waits]:
                        ctr += 1
                        nop = mybir.InstNoOp(
                            name=f"wsplit-{ctr}-{ins.name}", ins=[], outs=[])
                        nop.engine = ins.engine
                        nop.sync_info = bass_rust.SyncInfo(
                            on_wait=[w], on_update=[])
                        nc.register_instruction(nop, overwrite=True)
                        out.append(nop)
                        changed = True
                    ins.sync_info = bass_rust.SyncInfo(
                        on_wait=waits[-max_waits:], on_update=list(si.on_update))
                out.append(ins)
            if changed:
                blk.instructions = out


def _build_program():
    f32 = mybir.dt.float32
    bf16 = mybir.dt.bfloat16
    Exp = mybir.ActivationFunctionType.Exp
    Ln = mybir.ActivationFunctionType.Ln
    Ident = mybir.ActivationFunctionType.Identity

    nc = bass.Bass("TRN2", target_bir_lowering=False, debug=False,
                   num_devices=1)
    xb_d = nc.dram_tensor("xb", [D, HW], bf16, kind="ExternalInput").ap()
    xq_d = nc.dram_tensor("xq", [D, SLC], bf16, kind="ExternalInput").ap()
    xtf_d = nc.dram_tensor("xtf", [128, NH * NT * D], bf16,
                           kind="ExternalInput").ap()
    bof_d = nc.dram_tensor("bof", [128, NH * NT * 16], bf16,
                           kind="ExternalInput").ap()
    seb_d = nc.dram_tensor("seb", [128, 8 * 128], bf16,
                           kind="ExternalInput").ap()
    sc_d = nc.dram_tensor("sc", [128, 8], f32, kind="ExternalInput").ap()
    out_d = nc.dram_tensor("out", [D, SLC], bf16,
                           kind="ExternalOutput").ap()

    with tile.TileContext(nc) as tc:
        with (
            tc.tile_pool(name="cn", bufs=1) as cn,
            tc.tile_pool(name="ew", bufs=52) as ew,
            tc.tile_pool(name="pp", bufs=16) as pp,
            tc.tile_pool(name="zl", bufs=4) as zlp,
            tc.tile_pool(name="zi", bufs=4) as zip_,
            tc.tile_pool(name="ob", bufs=2) as obp,
            tc.tile_pool(name="ps_g", bufs=1, space="PSUM") as ps_g,
            tc.tile_pool(name="ps_zf", bufs=1, space="PSUM") as ps_zf,
            tc.tile_pool(name="ps_zb", bufs=2, space="PSUM") as ps_zb,
            tc.tile_pool(name="ps_av", bufs=1, space="PSUM") as ps_av,
        ):
            XB = cn.tile([D, HW], bf16, tag="XB")
            XQ = cn.tile([D, SLC], bf16, tag="XQ")
            XTF = cn.tile([128, NH * NT * D], bf16, tag="XTF")
            BOF = cn.tile([128, NH * NT * 16], bf16, tag="BOF")
            SEB = cn.tile([128, 8 * 128], bf16, tag="SEB")
            SC = cn.tile([128, 8], f32, tag="SC")
            for tl, dr in ((SC, sc_d), (XB, xb_d), (XQ, xq_d),
                           (BOF, bof_d), (SEB, seb_d), (XTF, xtf_d)):
                nc.sync.dma_start(tl[:], dr[:])

            state = {}

            def emit_A(jc, og):
                jkoff = jc * JKC
                Epairs = {}
                for pq in range(4):
                    t0 = 8 * og + 2 * pq
                    g = ps_g.tile([128, 2 * JKC], f32, tag="g")
                    for k in range(2):
                        nc.tensor.matmul(
                            g[:, k * JKC:(k + 1) * JKC],
                            XB[:, (t0 + k) * 128:(t0 + k + 1) * 128],
                            XQ[:, jkoff:jkoff + JKC],
                            start=True, stop=True)
                    for h in range(NH):
                        E = ew.tile([128, 2 * JKC], bf16, tag="E",
                                    name=f"E{jc}_{og}_{pq}_{h}")
                        nc.scalar.activation(E[:], g[:], Exp,
                                             bias=SC[:, 6:7],
                                             scale=SC[:, h:h + 1])
                        Epairs[(h, pq)] = E
                state[(jc, og)] = [Epairs, None]

            def emit_M(jc, og):
                Epairs = state[(jc, og)][0]
                zf = ps_zf.tile([128, JKC], f32, tag="zf",
                                name=f"zf{jc}_{og}")
                # MMZ: one dense batch of 8 x 4-concurrent bursts
                for pq in range(4):
                    for k in range(2):
                        tp = 2 * pq + k
                        t = 8 * og + tp
                        for h in range(NH):
                            nc.tensor.matmul(
                                zf[32 * h:32 * h + 16, :],
                                BOF[:, (h * NT + t) * 16:
                                    (h * NT + t + 1) * 16],
                                Epairs[(h, pq)][:, k * JKC:(k + 1) * JKC],
                                start=(tp == 0), stop=(tp == 7),
                                tile_position=(0, 32 * h))
                zl = zlp.tile([128, JKC], f32, tag="zl",
                              name=f"zl{jc}_{og}")
                nc.scalar.activation(zl[:], zf[:], Ln)
                zib = zip_.tile([128, JKC], bf16, tag="zib",
                                name=f"zib{jc}_{og}")
                nc.scalar.activation(zib[:], zl[:], Exp, scale=-1.0,
                                     bias=SC[:, 4:5])
                state[(jc, og)][1] = zib

            def emit_C(jc, og, av, mmav_n, half):
                Epairs, zib = state[(jc, og)]
                if half == 1:
                    state.pop((jc, og))
                for pq in (0, 1, 2) if half == 0 else (3,):
                    for hp in range(2):
                        zbs = []
                        for hh in range(2):
                            h = 2 * hp + hh
                            zbp = ps_zb.tile([128, 2 * JKC], f32,
                                             tag="zb")
                            for k in range(2):
                                tp = 2 * pq + k
                                nc.tensor.matmul(
                                    zbp[:, k * JKC:(k + 1) * JKC],
                                    SEB[32 * h:32 * h + 16,
                                        tp * 128:(tp + 1) * 128],
                                    zib[32 * h:32 * h + 16, :],
                                    start=True, stop=True,
                                    tile_position=(32 * h, 0))
                            zbs.append(zbp)
                        for hh in range(2):
                            h = 2 * hp + hh
                            p = pp.tile([128, 2 * JKC], bf16, tag="p",
                                        name=f"p{jc}_{og}_{pq}_{h}")
                            nc.vector.tensor_mul(
                                p[:], Epairs[(h, pq)][:], zbs[hh][:])
                            for k in range(2):
                                t = 8 * og + 2 * pq + k
                                nc.tensor.matmul(
                                    av[:],
                                    XTF[:, (h * NT + t) * D:
                                        (h * NT + t + 1) * D],
                                    p[:, k * JKC:(k + 1) * JKC],
                                    start=(mmav_n[0] == 0),
                                    stop=(mmav_n[0] == 2 * NT * NH - 1))
                                mmav_n[0] += 1

            # two-og software stagger: A(i) emitted before C(i-2)
            STAG = 2
            steps = [(jc, og) for jc in range(NJC) for og in range(NOG)]
            avs = {}
            for i in range(len(steps) + STAG):
                if i < len(steps):
                    jc, og = steps[i]
                    if og == 0:
                        avs[jc] = (ps_av.tile([D, JKC], f32, tag="av",
                                              name=f"av{jc}"), [0])
                    emit_A(jc, og)
                if i >= STAG:
                    cjc, cog = steps[i - STAG]
                    emit_C(cjc, cog, avs[cjc][0], avs[cjc][1], 0)
                if i < len(steps):
                    emit_M(*steps[i])
                if i >= STAG:
                    cjc, cog = steps[i - STAG]
                    emit_C(cjc, cog, avs[cjc][0], avs[cjc][1], 1)
                    if cog == NOG - 1:
                        ob = obp.tile([D, JKC], bf16, tag="ob",
                                      name=f"ob{cjc}")
                        nc.scalar.activation(ob[:], avs[cjc][0][:], Ident,
                                             bias=SC[0:D, 5:6])
                        nc.sync.dma_start(
                            out_d[:, cjc * JKC:(cjc + 1) * JKC], ob[:])

    _split_excess_waits(nc)
    return nc


_NC = None


def _get_program():
    global _NC
    if _NC is None:
        _NC = _build_program()
    return _NC


def _make_in_maps(x, wq, bq, wk, bk, wv, bv):
    x = np.asarray(x, dtype=np.float32)
    x2 = x.reshape(B, D, HW)
    wq, bq, wk, bk, wv, bv = [np.asarray(a, dtype=np.float32)
                              for a in (wq, bq, wk, bk, wv, bv)]

    # selector for SEL broadcast: SEB[32h + 2tp + g, tp*128 + 64g : +64] = 1
    seb = np.zeros((128, 8 * 128), dtype=BF16)
    for h in range(NH):
        for tp in range(8):
            for g in range(2):
                seb[32 * h + 2 * tp + g,
                    tp * 128 + 64 * g: tp * 128 + 64 * g + 64] = BF16(1.0)

    sc = np.zeros((128, 8), dtype=np.float32)
    for h in range(NH):
        sc[:, h] = SCALE * wk[h] * wq[h]          # a_h (exp scale)
        sc[32 * h:32 * h + 32, 4] = np.log(np.abs(wv[h]) + 1e-30)
    sc[:, 5] = 64.0 * bv.sum()
    sc[:, 6] = EBIAS

    per_batch = []
    for b in range(B):
        xb = x2[b]                                 # [D, HW]
        s_lm = xb.sum(axis=0)                      # [HW]
        xt = xb.reshape(D, NT, 128).transpose(2, 1, 0)   # [128, NT, D]
        xtf = np.zeros((128, NH * NT * D), dtype=np.float32)
        bof = np.zeros((128, NH * NT * 16), dtype=np.float32)
        for h in range(NH):
            c_h = (SCALE * wk[h] * bq[h]) * s_lm   # [HW]
            f_h = np.exp(c_h.astype(np.float64)).astype(np.float32)
            f_t = f_h.reshape(NT, 128).T           # [128, NT]
            # xtf block: [128, NT*D], scaled per (partition p, tile t)
            blk = xt * (np.sign(wv[h]) * f_t)[:, :, None]  # [128, NT, D]
            xtf[:, h * NT * D:(h + 1) * NT * D] = blk.reshape(128, NT * D)
            for t in range(NT):
                tp = t % 8
                for g in range(2):
                    # BOF[:, (h*NT+t)*16 + 2*tp + g] over partitions g*64+j
                    col = (h * NT + t) * 16 + 2 * tp + g
                    bof[g * 64:(g + 1) * 64, col] = \
                        f_h[t * 128 + g * 64: t * 128 + (g + 1) * 64]
        per_batch.append({
            "xb": xb.astype(BF16),
            "xtf": xtf.astype(BF16),
            "bof": bof.astype(BF16),
        })

    in_maps = []
    for c in range(NCORES):
        b, sl = divmod(c, NH)
        pb = per_batch[b]
        in_maps.append({
            "xb": pb["xb"],
            "xq": np.ascontiguousarray(
                pb["xb"][:, sl * SLC:(sl + 1) * SLC]),
            "xtf": pb["xtf"],
            "bof": pb["bof"],
            "seb": seb,
            "sc": sc,
        })
    return in_maps


def kernel(x, wq, bq, wk, bk, wv, bv):
    nc = _get_program()
    in_maps = _make_in_maps(x, wq, bq, wk, bk, wv, bv)
    res = run_bass_kernel_spmd(nc, in_maps, core_ids=list(range(NCORES)))
    out = np.zeros((B, 1, D, 64, 64), dtype=np.float32)
    for c in range(NCORES):
        b, sl = divmod(c, NH)
        out[b, 0].reshape(D, HW)[:, sl * SLC:(sl + 1) * SLC] = \
            res.results[c]["out"].astype(np.float32)
    return out
